# revision 1
# baseline (speedup 1.0000x reference)
"""Trainium2 Bass kernel for nn_CustomLoss (2-Wasserstein-style Gaussian loss).

loss = mean((mu_p-mu_t)^2) + tr(Cp) + tr(Ct) + 2*tr(sqrtm(S2 @ Ct @ S2)),
       S2 = sqrtm(Cp),  d = 2048, packed inputs (4, 2100224), row 0 used.

Device algorithm: two scaled coupled Newton-Schulz sqrt chains in fp32r
(TensorEngine full rate), 8-way row-sharded across the NeuronCores with
AllGather (full operands for streaming) + AllToAll (column-slice delivery for
the stationary operand, avoiding core-dependent addressing in the SPMD
program). Scalar normalizers and the per-iteration scaling schedule are
host-side; the schedule is input-independent so one NEFF serves all inputs.
"""
import numpy as np

import concourse.bass as bass
import concourse.mybir as mybir
import concourse.tile as tile
from concourse.bass_utils import run_bass_kernel_spmd
from concourse.masks import make_identity

# Disable the walrus-embedded BIR simulator: ~4x faster NEFF compiles.
import concourse.bass_utils as _bu
if not getattr(_bu, "_nobirsim_patched", False):
    _orig_bvo = _bu.bir_verify_and_optimise

    def _bvo_fast(tmpdir, inp="bir.json", outp="file.neff", arch=None, *, dve_root=None):
        orig_run = _bu.run_command

        def patched_run(argv, **kw):
            argv = [a.replace("--enable-birsim=true", "--enable-birsim=false")
                    if isinstance(a, str) else a for a in argv]
            return orig_run(argv, **kw)

        _bu.run_command = patched_run
        try:
            return _orig_bvo(tmpdir, inp, outp, arch, dve_root=dve_root)
        finally:
            _bu.run_command = orig_run

    _bu.bir_verify_and_optimise = _bvo_fast
    _bu._nobirsim_patched = True

# ----------------------------------------------------------------------------
# config
D = 2048
NC = 8
SH = D // NC          # 256 rows per core
P = 128
KT = D // P           # 16 k-tiles
MB = SH // P          # 2 m-blocks per shard
NB = D // 512         # 4 n-blocks
CH = 2                # k-tiles per stream chunk
_TAG_BUFS = {"ostag": 2, "tstag": 2, "zstag": 1, "rstream": 2, "lhsT": 3}
EPS = 1e-4            # ridge (normalized units)
QCAP = 2.5            # max scaled eigenvalue (stability margin)
K1 = 10               # NS1 iterations (incl. cheap iter 1) + half-step
K2 = 12               # NS2 iterations (incl. cheap iter 1) + trace correction
F32 = mybir.dt.float32
F32R = mybir.dt.float32r
AF = mybir.ActivationFunctionType
ALU = mybir.AluOpType

_BUILD_CACHE = {}


# ----------------------------------------------------------------------------
# host: schedule
def _f(q):
    return q * (3.0 - q) ** 2 / 4.0


def _balance_s(a, b, qcap):
    """s with f(s*a) = f(s*b), s*b <= qcap, via bisection."""
    s_hi = min(qcap, 2.9999) / b
    g = lambda s: _f(s * a) - _f(s * b)
    if g(s_hi) <= 0:
        return s_hi
    lo, hi = 1e-12, s_hi
    for _ in range(80):
        mid = 0.5 * (lo + hi)
        if g(mid) > 0:
            hi = mid
        else:
            lo = mid
    return 0.5 * (lo + hi)


def make_schedule(delta, b0, iters, qcap=QCAP):
    a, b = delta, b0
    out = []
    for _ in range(iters):
        s = 1.0 if a > 0.99 * b else _balance_s(a, b, qcap)
        mu = np.sqrt(s)
        out.append((1.5 * mu, -0.5 * mu ** 3))   # (alpha, beta): T = a*I + b*P
        qa, qb = s * a, s * b
        vals = [_f(qa), _f(qb)]
        b = 1.0 if qa <= 1.0 <= qb else max(vals)
        a = min(vals)
    return out


# ----------------------------------------------------------------------------
# host: input prep
def _unpack_row(v):
    mu = v[:D].astype(np.float64)
    tri = v[D:]
    C = np.zeros((D, D), np.float32)
    iu, ju = np.triu_indices(D)
    C[iu, ju] = tri
    C[ju, iu] = tri
    return mu, C


def _power_iter_sym(C, iters=60):
    rng = np.random.default_rng(12345)
    x = rng.standard_normal(D)
    C64 = C.astype(np.float64)
    lam = 1.0
    for _ in range(iters):
        y = C64 @ x
        lam = np.linalg.norm(y)
        x = y / lam
    return float(lam)


def _power_iter_prod(Cp, Ct, iters=60):
    rng = np.random.default_rng(54321)
    x = rng.standard_normal(D)
    Cp64 = Cp.astype(np.float64)
    Ct64 = Ct.astype(np.float64)
    lam = 1.0
    for _ in range(iters):
        y = Cp64 @ (Ct64 @ x)
        lam = np.linalg.norm(y)
        x = y / lam
    return float(lam)


# ----------------------------------------------------------------------------
# walrus workaround: this build allows only ONE sync-wait per instruction
class PatchedTileContext(tile.TileContext):
    def _drain_and_barrier(self, tick_clock, wait_clock):
        from concourse.vector_clock import ScopedClock

        probe = self.nc.sync.nop(nofuse=True)
        wait_clock.add_sem_waits(
            probe.ins, ScopedClock({None: tick_clock.global_clock})
        )
        si = probe.ins.sync_info
        waits = list(si.on_wait) if si is not None else []
        if len(waits) > 1:
            si.on_wait = [waits[0]]
            for w in waits[1:]:
                n2 = self.nc.sync.nop(nofuse=True)
                si2 = n2.ins.sync_info
                if si2 is None:
                    n2.ins.sync_info = mybir.SyncInfo(on_wait=[w], on_update=[])
                else:
                    si2.on_wait = [w]
        self.nc.sync.drain()
        self.nc.all_engine_barrier()
        assert self.sems is not None
        popped = self.nc._tile_sem_poison_stack.pop()
        assert popped is self._sem_poison
        self.nc.clear_and_free_semaphores(list(self.sems.allocated().values()))
        self.nc.all_engine_barrier()


def legalize_single_wait(nc):
    uid = 0
    for fn in nc.m.functions:
        for blk in fn.blocks:
            il = blk.instructions
            if not any(
                i.sync_info is not None and len(i.sync_info.on_wait) > 1 for i in il
            ):
                continue
            new = []
            for ins in il:
                si = ins.sync_info
                waits = list(si.on_wait) if si is not None else []
                if len(waits) > 1:
                    si.on_wait = [waits[-1]]
                    for w in waits[:-1]:
                        nop = mybir.InstNoOp(
                            name=f"legalize-wait-{uid}",
                            engine=ins.engine,
                            sync_info=mybir.SyncInfo(on_wait=[w], on_update=[]),
                        )
                        uid += 1
                        new.append(nop)
                new.append(ins)
            blk.instructions = new


# ----------------------------------------------------------------------------
# device program builder
class _B:
    """Builder state."""

    def __init__(self, nc, tc, dram, sb, psum):
        self.nc, self.tc = nc, tc
        self.dram, self.sb, self.psum = dram, sb, psum
        self.uid = 0
        self.ident = None    # [P, P] identity f32
        self.epsrow = None   # [P, MB, D] eps*I row slab (per-core input)

    def u(self, s):
        self.uid += 1
        return f"{s}_{self.uid}"


def _stream_view(full_ap):
    """[D, D] dram AP -> [P, NCH, CH, D] chunked k-tile stream view."""
    return full_ap.rearrange("(ch kb p) n -> p ch kb n", p=P, kb=CH)


def _lhsT_view(a2a_ap):
    """[D, SH] dram AP (A2A out, flat) -> [P, KT, SH]."""
    return a2a_ap.rearrange("(k p) m -> p k m", p=P)


def _mm_shard(b: _B, lhsT_sb, rhs_chunks, scale, eps_coef, tag="ostag"):
    """out_stag[P, MB, D] = (lhsT^T @ rhs) * scale (+ eps_coef * epsrow).

    lhsT_sb: [P, KT, SH] f32 sbuf; rhs_chunks: [P, NCHUNK, CH, D] dram view.
    scale: float or AP. eps_coef: None or float g (adds g * epsrow).
    """
    nc = b.nc
    stag = b.sb.tile([P, MB, D], F32R, tag=tag, name=b.u(tag), bufs=_TAG_BUFS[tag])
    ps = [
        b.psum.tile([P, 512], F32, tag="mmps", name=b.u("ps"))
        for _ in range(MB * NB)
    ]
    for ch in range(KT // CH):
        rt = b.sb.tile([P, CH, D], F32R, tag="rstream", name=b.u("rt"), bufs=_TAG_BUFS["rstream"])
        nc.sync.dma_start(out=rt[:], in_=rhs_chunks[:, ch])
        for kk in range(CH):
            k = ch * CH + kk
            for m in range(MB):
                for n in range(NB):
                    nc.tensor.matmul(
                        ps[m * NB + n][:],
                        lhsT_sb[:, k, m * P:(m + 1) * P],
                        rt[:, kk, n * 512:(n + 1) * 512],
                        start=(k == 0),
                        stop=(k == KT - 1),
                    )
    for m in range(MB):
        for n in range(NB):
            if eps_coef is not None:
                # add (eps_coef/scale) * epsrow into psum pre-eviction so the
                # scaled eviction yields  scale*psum + eps_coef*epsrow
                nc.vector.scalar_tensor_tensor(
                    ps[m * NB + n][:],
                    b.epsrow[:, m, n * 512:(n + 1) * 512],
                    float(eps_coef) / _scale_const(scale),
                    ps[m * NB + n][:],
                    ALU.mult,
                    ALU.add,
                )
            nc.scalar.activation(
                stag[:, m, n * 512:(n + 1) * 512],
                ps[m * NB + n][:],
                AF.Copy,
                scale=scale,
            )
    return stag


def _scale_const(scale):
    assert isinstance(scale, (int, float)), "eps_coef requires constant scale"
    return float(scale)


def _transpose_shard(b: _B, stag):
    """[P, MB, D] staging (rows shard of X) -> [P, KT, SH] = X^T[:, shard cols]."""
    nc = b.nc
    tt = b.sb.tile([P, KT, SH], F32R, tag="lhsT", name=b.u("tt"), bufs=_TAG_BUFS["lhsT"])
    for k in range(KT):
        for m in range(MB):
            tp = b.psum.tile([P, 512], F32R, tag="mmps", name=b.u("tps"))
            nc.tensor.transpose(
                tp[:, 0:P], stag[:, m, k * P:(k + 1) * P], b.ident[:]
            )
            nc.scalar.copy(tt[:, k, m * P:(m + 1) * P], tp[:, 0:P])
    return tt


def _load_lhsT(b: _B, dram_flat_ap):
    """DMA [D, SH] dram -> [P, KT, SH] sbuf."""
    t = b.sb.tile([P, KT, SH], F32R, tag="lhsT", name=b.u("lh"), bufs=_TAG_BUFS["lhsT"])
    b.nc.sync.dma_start(out=t[:], in_=_lhsT_view(dram_flat_ap))
    return t


def _bounce_and_gather(b: _B, stag, want_a2a, name):
    """Write staging to DRAM, AllGather full (+ optionally AllToAll col-slice).

    Returns (full_dram_ap [D, D], a2a_out_ap [D, SH] or None).
    """
    nc = b.nc
    bounce = b.dram.tile([SH, D], F32R, name=b.u(f"bn_{name}"), tag="d_bn", bufs=4)
    nc.gpsimd.dma_start(
        out=bounce[:].rearrange("(m p) n -> p m n", p=P), in_=stag[:]
    )
    full = b.dram.tile([D, D], F32R, name=b.u(f"fl_{name}"), addr_space="Shared", tag="d_fl", bufs=4)
    nc.gpsimd.collective_compute(
        "AllGather",
        ALU.bypass,
        replica_groups=[list(range(NC))],
        ins=[bounce[:]],
        outs=[full[:]],
    )
    a2a_out = None
    if want_a2a:
        a2a_in = b.dram.tile([NC, SH, SH], F32R, name=b.u(f"ai_{name}"), tag="d_ai", bufs=4)
        for j in range(NC):
            nc.gpsimd.dma_start(
                out=a2a_in[j].rearrange("(m p) n -> p m n", p=P),
                in_=stag[:, :, j * SH:(j + 1) * SH],
            )
        a2a_out = b.dram.tile([NC * SH, SH], F32R, name=b.u(f"ao_{name}"), tag="d_ao", bufs=4)
        nc.gpsimd.collective_compute(
            "AllToAll",
            ALU.bypass,
            replica_groups=[list(range(NC))],
            ins=[a2a_in[:]],
            outs=[a2a_out[:]],
        )
    return full[:], (a2a_out[:] if a2a_out is not None else None)


def _ns_chain(b: _B, a_col_lhsT_sb, a_row_stag, sched, name):
    """Run a scaled NS chain. Inputs:
      a_col_lhsT_sb: [P, KT, SH] sbuf = A[:, shard cols]  (lhsT of A)
      a_row_stag:    [P, MB, D] sbuf = A[shard rows, :]   (row slab of A)
    Returns dict with Yfull, Zfull (dram APs), Y_a2a, Z_a2a, Y_stag (sbuf).
    """
    nc = b.nc
    al0, be0 = sched[0]
    # iter 1: T0 = al0*I + be0*A (sharded, elementwise); Z1 = T0; Y1 = A @ T0
    t0f = b.sb.tile([P, MB, D], F32, tag="f32tmp", name=b.u("t0f"), bufs=1)
    t0 = b.sb.tile([P, MB, D], F32R, tag="ostag", name=b.u("t0"), bufs=_TAG_BUFS["ostag"])
    for m in range(MB):
        nc.scalar.mul(t0f[:, m, :], a_row_stag[:, m, :].bitcast(F32), float(be0))
        nc.vector.scalar_tensor_tensor(
            t0f[:, m, :], b.epsrow[:, m, :], float(al0 / EPS),
            t0f[:, m, :], ALU.mult, ALU.add,
        )
        nc.scalar.copy(t0[:, m, :], t0f[:, m, :])
    t0_full, t0_a2a = _bounce_and_gather(b, t0, True, f"{name}t0")
    y_stag = _mm_shard(b, a_col_lhsT_sb, _stream_view(t0_full), 1.0, None)
    y_full, y_a2a = _bounce_and_gather(b, y_stag, True, f"{name}y1")
    st = dict(Yfull=y_full, Y_a2a=y_a2a, Zfull=t0_full, Z_a2a=t0_a2a, Y_stag=y_stag)

    for it in range(1, len(sched)):
        al, be = sched[it]
        lh_z = _get_lhsT(b, st, "Z")
        lh_y = _get_lhsT(b, st, "Y")
        # P = Z @ Y ; T = al*I + be*P  (keep T staging for local transpose)
        t_stag = _mm_shard(b, lh_z, _get_stream(b, st, "Y"), float(be), al / EPS,
                           tag="tstag")
        t_full, _ = _bounce_and_gather(b, t_stag, False, f"{name}t{it}")
        # Z' = T @ Z : lhsT = T^T[:, shard] = transpose of own T staging
        lh_tt = _transpose_shard(b, t_stag)
        z_stag = _mm_shard(b, lh_tt, _get_stream(b, st, "Z"), 1.0, None,
                           tag="zstag")
        # Y' = Y @ T
        y_stag = _mm_shard(b, lh_y, _stream_view(t_full), 1.0, None)
        # batched gather of (Y', Z')
        bounce = b.dram.tile([2 * SH, D], F32R, name=b.u("bnyz"), tag="d_bnyz", bufs=4)
        nc.gpsimd.dma_start(
            out=bounce[:].rearrange("(t m p) n -> t p m n", t=2, p=P)[0],
            in_=y_stag[:])
        nc.gpsimd.dma_start(
            out=bounce[:].rearrange("(t m p) n -> t p m n", t=2, p=P)[1],
            in_=z_stag[:])
        full = b.dram.tile([NC * 2 * SH, D], F32R, name=b.u("flyz"),
                           addr_space="Shared", tag="d_flyz", bufs=4)
        nc.gpsimd.collective_compute(
            "AllGather", ALU.bypass, replica_groups=[list(range(NC))],
            ins=[bounce[:]], outs=[full[:]],
        )
        a2a_in = b.dram.tile([NC, 2, SH, SH], F32R, name=b.u("aiyz"), tag="d_aiyz", bufs=4)
        for j in range(NC):
            nc.gpsimd.dma_start(
                out=a2a_in[j, 0].rearrange("(m p) n -> p m n", p=P),
                in_=y_stag[:, :, j * SH:(j + 1) * SH])
            nc.gpsimd.dma_start(
                out=a2a_in[j, 1].rearrange("(m p) n -> p m n", p=P),
                in_=z_stag[:, :, j * SH:(j + 1) * SH])
        a2a_out = b.dram.tile([NC, 2, SH, SH], F32R, name=b.u("aoyz"), tag="d_aoyz", bufs=4)
        nc.gpsimd.collective_compute(
            "AllToAll", ALU.bypass, replica_groups=[list(range(NC))],
            ins=[a2a_in[:]], outs=[a2a_out[:]],
        )
        # views: full rows = (c, t, m p); Y = t 0, Z = t 1
        fv = full[:].rearrange("(c t kb p) n -> t p c kb n", t=2, kb=CH, p=P)
        av = a2a_out[:].rearrange("s t (kb p) m -> t p s kb m", kb=CH, p=P)
        st = dict(
            Yfull=fv[0], Zfull=fv[1],           # [P, NC, CH, D] chunk views
            Y_a2a=av[0], Z_a2a=av[1],           # [P, s, kb, SH] 4d lhsT views
            Y_stag=y_stag, Z_stag=z_stag,
            chunked=True,
        )
    return st


def _load_lhsT4(b: _B, view4):
    """DMA [P, s, kb, SH] 4d view -> [P, KT, SH] sbuf (k = s*CH + kb)."""
    t = b.sb.tile([P, KT, SH], F32R, tag="lhsT", name=b.u("lh4"), bufs=_TAG_BUFS["lhsT"])
    for s in range(NC):
        b.nc.sync.dma_start(
            out=t[:, s * CH:(s + 1) * CH, :], in_=view4[:, s]
        )
    return t


def _get_lhsT(b, st, key):
    v = st[f"{key}_a2a"]
    if st.get("chunked"):
        return _load_lhsT4(b, v)
    return _load_lhsT(b, v)


def _get_stream(b, st, key):
    v = st[f"{key}full"]
    if st.get("chunked"):
        return v
    return _stream_view(v)


def build_device_program(k1, k2, repeat=1):
    sched1 = make_schedule(EPS, 1.0 + EPS, k1)
    sched2 = make_schedule(EPS, 1.0 + EPS, k2)

    nc = bass.Bass(num_devices=NC)
    with PatchedTileContext(nc) as tc:
        with tc.tile_pool(name="dram", bufs=1, space="DRAM") as dram, \
             tc.tile_pool(name="sb", bufs=1) as sb_const, \
             tc.tile_pool(name="sbw", bufs=3) as sbw, \
             tc.tile_pool(name="psum", bufs=8, space="PSUM") as psum:

            b = _B(nc, tc, dram, sbw, psum)

            # --- inputs
            a1col = dram.tile([D, SH], F32R, kind="ExternalInput", name="a1col", uniquify=False)
            a1row = dram.tile([SH, D], F32, kind="ExternalInput", name="a1row", uniquify=False)
            ctcol = dram.tile([D, SH], F32R, kind="ExternalInput", name="ctcol", uniquify=False)
            epsrow_d = dram.tile([SH, D], F32, kind="ExternalInput", name="epsrow", uniquify=False)
            invc2_d = dram.tile([P, 1], F32, kind="ExternalInput", name="invc2", uniquify=False)
            partials_d = dram.tile([P, 8], F32, kind="ExternalOutput", name="partials", uniquify=False)

            # --- constants resident in SBUF
            ident_f = sb_const.tile([P, P], F32, name="ident_f", uniquify=False)
            make_identity(nc, ident_f[:])
            ident = sb_const.tile([P, P], F32R, name="ident", uniquify=False)
            nc.scalar.copy(ident[:], ident_f[:])
            b.ident = ident
            epsrow = sb_const.tile([P, MB, D], F32, name="epsrow_sb", uniquify=False)
            nc.sync.dma_start(out=epsrow[:], in_=epsrow_d[:].rearrange("(m p) n -> p m n", p=P))
            b.epsrow = epsrow
            invc2 = sb_const.tile([P, 1], F32, name="invc2_sb", uniquify=False)
            nc.sync.dma_start(out=invc2[:], in_=invc2_d[:])
            part = sb_const.tile([P, 8], F32, name="part_sb", uniquify=False)
            b.part = part

            for _rep in range(repeat):
                _emit_pipeline(b, nc, sched1, sched2, a1col, a1row, ctcol,
                               epsrow, invc2, partials_d)

    legalize_single_wait(nc)
    return nc


def _emit_pipeline(b, nc, sched1, sched2, a1col, a1row, ctcol, epsrow, invc2,
                   partials_d):
    if True:
        if True:
            # --- NS1 on A1 (uploaded: Cp/c1 + eps I)
            a1c_sb = _load_lhsT(b, a1col[:])
            a1r_sb = b.sb.tile([P, MB, D], F32, tag="ostag", name="a1r_sb", bufs=_TAG_BUFS["ostag"])
            nc.sync.dma_start(out=a1r_sb[:], in_=a1row[:].rearrange("(m p) n -> p m n", p=P))
            st1 = _ns_chain(b, a1c_sb, a1r_sb, sched1, "n1")

            # --- NS1 half-step: S = Y*(1.5 I - 0.5 Z Y)
            lh_z = _get_lhsT(b, st1, "Z")
            lh_y = _get_lhsT(b, st1, "Y")
            tp_stag = _mm_shard(b, lh_z, _get_stream(b, st1, "Y"), -0.5, 1.5 / EPS,
                                tag="tstag")
            tp_full, _ = _bounce_and_gather(b, tp_stag, False, "half")
            s_stag = _mm_shard(b, lh_y, _stream_view(tp_full), 1.0, None)
            s_full, s_a2a = _bounce_and_gather(b, s_stag, True, "sfin")

            # --- middle: V = (Ct @ S)/c2 ; A2 = S @ V + eps I
            ct_sb = _load_lhsT(b, ctcol[:])
            v_stag = _mm_shard(b, ct_sb, _stream_view(s_full), invc2[:, 0:1],
                               None, tag="tstag")
            v_full, _ = _bounce_and_gather(b, v_stag, False, "vmid")
            lh_s = _load_lhsT(b, s_a2a)
            a2_stag = _mm_shard(b, lh_s, _stream_view(v_full), 1.0, 1.0)
            # A2: only A2A needed (lhsT for NS2 iter1); row slab is local staging
            a2a_in = b.dram.tile([NC, SH, SH], F32R, name=b.u("ai_a2"), tag="d_ai", bufs=4)
            for j in range(NC):
                nc.gpsimd.dma_start(
                    out=a2a_in[j].rearrange("(m p) n -> p m n", p=P),
                    in_=a2_stag[:, :, j * SH:(j + 1) * SH])
            a2_a2a = b.dram.tile([NC * SH, SH], F32R, name=b.u("ao_a2"), tag="d_ao", bufs=4)
            nc.gpsimd.collective_compute(
                "AllToAll", ALU.bypass, replica_groups=[list(range(NC))],
                ins=[a2a_in[:]], outs=[a2_a2a[:]],
            )
            a2c_sb = _load_lhsT(b, a2_a2a[:])

            # --- NS2
            st2 = _ns_chain(b, a2c_sb, a2_stag, sched2, "n2")

            # --- trace stage: U2 = Y2 @ Z2 (staging only)
            lh_y2 = _get_lhsT(b, st2, "Y")
            u2_stag = _mm_shard(b, lh_y2, _get_stream(b, st2, "Z"), 1.0, None,
                                tag="tstag")
            y2_stag = st2["Y_stag"]
            part = b.part
            nc.gpsimd.memset(part[:], 0.0)
            tmp = b.sb.tile([P, MB, D], F32, tag="f32tmp", name=b.u("tmp"), bufs=1)
            for m in range(MB):
                nc.vector.tensor_mul(
                    tmp[:, m, :], y2_stag[:, m, :].bitcast(F32),
                    u2_stag[:, m, :].bitcast(F32))
                nc.vector.tensor_reduce(
                    part[:, m:m + 1], tmp[:, m, :], mybir.AxisListType.X, ALU.add)
                nc.vector.tensor_mul(
                    tmp[:, m, :], y2_stag[:, m, :].bitcast(F32), epsrow[:, m, :])
                nc.vector.tensor_reduce(
                    part[:, 2 + m:3 + m], tmp[:, m, :], mybir.AxisListType.X, ALU.add)
            nc.sync.dma_start(out=partials_d[:], in_=part[:])


# ----------------------------------------------------------------------------
# host golden model (mirrors device pipeline exactly, fp32, no hw noise)
def golden_loss(predictions, targets, k1=K1, k2=K2):
    mu_p, Cp = _unpack_row(predictions[0])
    mu_t, Ct = _unpack_row(targets[0])
    c1 = _power_iter_sym(Cp) * 1.02
    c2 = _power_iter_prod(Cp, Ct) * 1.05 / c1
    I = np.eye(D, dtype=np.float32)
    A1 = (Cp / c1 + EPS * I).astype(np.float32)

    def chain(A, sched):
        al, be = sched[0]
        T0 = (al * I + be * A).astype(np.float32)
        Y, Z = A @ T0, T0
        for alk, bek in sched[1:]:
            Pm = Z @ Y
            T = alk * I + bek * Pm
            Y, Z = Y @ T, T @ Z
        return Y, Z

    Y1, Z1 = chain(A1, make_schedule(EPS, 1.0 + EPS, k1))
    S = Y1 @ (1.5 * I - 0.5 * (Z1 @ Y1))
    V = (Ct @ S) / c2
    A2 = (S @ V + EPS * I).astype(np.float32)
    Y2, Z2 = chain(A2, make_schedule(EPS, 1.0 + EPS, k2))
    U2 = Y2 @ Z2
    tr_corr = 1.5 * np.trace(Y2.astype(np.float64)) - 0.5 * float(
        np.sum(Y2.astype(np.float64) * U2.astype(np.float64)))
    tr_sqrtM = np.sqrt(c1 * c2) * tr_corr
    mu_term = float(np.mean((mu_p - mu_t) ** 2))
    return np.float32(mu_term + np.trace(Cp.astype(np.float64))
                      + np.trace(Ct.astype(np.float64)) + 2.0 * tr_sqrtM)


# ----------------------------------------------------------------------------
# entry point
def _get_program():
    key = (K1, K2)
    if key not in _BUILD_CACHE:
        _BUILD_CACHE[key] = build_device_program(K1, K2)
    return _BUILD_CACHE[key]


def kernel(predictions, targets):
    predictions = np.asarray(predictions)
    targets = np.asarray(targets)
    mu_p, Cp = _unpack_row(predictions[0])
    mu_t, Ct = _unpack_row(targets[0])

    c1 = _power_iter_sym(Cp) * 1.02
    c2 = _power_iter_prod(Cp, Ct) * 1.05 / c1

    I = np.eye(D, dtype=np.float32)
    A1 = (Cp / c1).astype(np.float32)
    A1[np.arange(D), np.arange(D)] += EPS

    nc = _get_program()

    in_maps = []
    for c in range(NC):
        sl = slice(c * SH, (c + 1) * SH)
        eps_row = np.zeros((SH, D), np.float32)
        eps_row[np.arange(SH), np.arange(c * SH, (c + 1) * SH)] = EPS
        in_maps.append({
            "a1col": np.ascontiguousarray(A1[:, sl]),
            "a1row": np.ascontiguousarray(A1[sl, :]),
            "ctcol": np.ascontiguousarray(Ct[:, sl]),
            "epsrow": eps_row,
            "invc2": np.full((P, 1), 1.0 / c2, np.float32),
        })

    res = run_bass_kernel_spmd(nc, in_maps, core_ids=list(range(NC)))
    parts = np.stack([r["partials"] for r in res.results])  # [NC, P, 8]
    syu = float(parts[:, :, 0:2].sum(dtype=np.float64))
    trY2 = float(parts[:, :, 2:4].sum(dtype=np.float64)) / EPS
    tr_corr = 1.5 * trY2 - 0.5 * syu
    tr_sqrtM = np.sqrt(c1 * c2) * tr_corr

    mu_term = float(np.mean((mu_p - mu_t) ** 2))
    loss = (mu_term + float(np.trace(Cp.astype(np.float64)))
            + float(np.trace(Ct.astype(np.float64))) + 2.0 * tr_sqrtM)
    return np.float32(loss)



# revision 3
# speedup vs baseline: 6.0670x; 6.0670x over previous
"""Trainium2 Bass kernel for nn_CustomLoss (2-Wasserstein-style Gaussian loss).

loss = mean((mu_p-mu_t)^2) + tr(Cp) + tr(Ct) + 2*tr(sqrtm(S2 @ Ct @ S2)),
       S2 = sqrtm(Cp),  d = 2048, packed inputs (4, 2100224), row 0 used.

Device algorithm: two scaled coupled Newton-Schulz sqrt chains in fp32r
(TensorEngine full rate), 8-way row-sharded across the NeuronCores with
AllGather (full operands for streaming) + AllToAll (column-slice delivery for
the stationary operand, avoiding core-dependent addressing in the SPMD
program). Scalar normalizers and the per-iteration scaling schedule are
host-side; the schedule is input-independent so one NEFF serves all inputs.
"""
import numpy as np

import concourse.bass as bass
import concourse.mybir as mybir
import concourse.tile as tile
from concourse.bass_utils import run_bass_kernel_spmd
from concourse.masks import make_identity

# Disable the walrus-embedded BIR simulator: ~4x faster NEFF compiles.
import concourse.bass_utils as _bu
if not getattr(_bu, "_nobirsim_patched", False):
    _orig_bvo = _bu.bir_verify_and_optimise

    def _bvo_fast(tmpdir, inp="bir.json", outp="file.neff", arch=None, *, dve_root=None):
        orig_run = _bu.run_command

        def patched_run(argv, **kw):
            argv = [a.replace("--enable-birsim=true", "--enable-birsim=false")
                    if isinstance(a, str) else a for a in argv]
            return orig_run(argv, **kw)

        _bu.run_command = patched_run
        try:
            return _orig_bvo(tmpdir, inp, outp, arch, dve_root=dve_root)
        finally:
            _bu.run_command = orig_run

    _bu.bir_verify_and_optimise = _bvo_fast
    _bu._nobirsim_patched = True

# ----------------------------------------------------------------------------
# config
D = 2048
NC = 8
SH = D // NC          # 256 rows per core
P = 128
KT = D // P           # 16 k-tiles
MB = SH // P          # 2 m-blocks per shard
NB = D // 512         # 4 n-blocks
CH = 2                # k-tiles per stream chunk
_TAG_BUFS = {"ostag": 2, "tstag": 2, "zstag": 1, "rstream": 2, "lhsT": 3}
EPS = 1e-4            # ridge (normalized units)
QCAP = 2.5            # max scaled eigenvalue (stability margin)
K1 = 10               # NS1 iterations (incl. cheap iter 1) + half-step
K2 = 12               # NS2 iterations (incl. cheap iter 1) + trace correction
F32 = mybir.dt.float32
F32R = mybir.dt.float32r
AF = mybir.ActivationFunctionType
ALU = mybir.AluOpType

_BUILD_CACHE = {}


# ----------------------------------------------------------------------------
# host: schedule
def _f(q):
    return q * (3.0 - q) ** 2 / 4.0


def _balance_s(a, b, qcap):
    """s with f(s*a) = f(s*b), s*b <= qcap, via bisection."""
    s_hi = min(qcap, 2.9999) / b
    g = lambda s: _f(s * a) - _f(s * b)
    if g(s_hi) <= 0:
        return s_hi
    lo, hi = 1e-12, s_hi
    for _ in range(80):
        mid = 0.5 * (lo + hi)
        if g(mid) > 0:
            hi = mid
        else:
            lo = mid
    return 0.5 * (lo + hi)


def make_schedule(delta, b0, iters, qcap=QCAP):
    a, b = delta, b0
    out = []
    for _ in range(iters):
        s = 1.0 if a > 0.99 * b else _balance_s(a, b, qcap)
        mu = np.sqrt(s)
        out.append((1.5 * mu, -0.5 * mu ** 3))   # (alpha, beta): T = a*I + b*P
        qa, qb = s * a, s * b
        vals = [_f(qa), _f(qb)]
        b = 1.0 if qa <= 1.0 <= qb else max(vals)
        a = min(vals)
    return out


# ----------------------------------------------------------------------------
# host: input prep
_IU_JU = None


def _unpack_row(v):
    global _IU_JU
    if _IU_JU is None:
        _IU_JU = np.triu_indices(D)
    iu, ju = _IU_JU
    v = np.asarray(v)
    mu = v[:D].astype(np.float64)
    tri = v[D:]
    C = np.zeros((D, D), np.float32)
    C[iu, ju] = tri
    C[ju, iu] = tri
    return mu, C


def _power_iter_sym(C, iters=10, nvec=4):
    """fp32 block power iteration; lower-bound estimate of lambda_max."""
    rng = np.random.default_rng(12345)
    X = rng.standard_normal((D, nvec)).astype(np.float32)
    lam = 1.0
    for _ in range(iters):
        Y = C @ X
        norms = np.sqrt((Y * Y).sum(axis=0))
        lam = float(norms.max())
        X = Y / np.maximum(norms, np.float32(1e-30))
    return lam


def _power_iter_prod(Cp, Ct, iters=10, nvec=4):
    rng = np.random.default_rng(54321)
    X = rng.standard_normal((D, nvec)).astype(np.float32)
    lam = 1.0
    for _ in range(iters):
        Y = Cp @ (Ct @ X)
        norms = np.sqrt((Y * Y).sum(axis=0))
        lam = float(norms.max())
        X = Y / np.maximum(norms, np.float32(1e-30))
    return lam


# ----------------------------------------------------------------------------
# walrus workaround: this build allows only ONE sync-wait per instruction
class PatchedTileContext(tile.TileContext):
    def _drain_and_barrier(self, tick_clock, wait_clock):
        from concourse.vector_clock import ScopedClock

        probe = self.nc.sync.nop(nofuse=True)
        wait_clock.add_sem_waits(
            probe.ins, ScopedClock({None: tick_clock.global_clock})
        )
        si = probe.ins.sync_info
        waits = list(si.on_wait) if si is not None else []
        if len(waits) > 1:
            si.on_wait = [waits[0]]
            for w in waits[1:]:
                n2 = self.nc.sync.nop(nofuse=True)
                si2 = n2.ins.sync_info
                if si2 is None:
                    n2.ins.sync_info = mybir.SyncInfo(on_wait=[w], on_update=[])
                else:
                    si2.on_wait = [w]
        self.nc.sync.drain()
        self.nc.all_engine_barrier()
        assert self.sems is not None
        popped = self.nc._tile_sem_poison_stack.pop()
        assert popped is self._sem_poison
        self.nc.clear_and_free_semaphores(list(self.sems.allocated().values()))
        self.nc.all_engine_barrier()


def legalize_single_wait(nc):
    uid = 0
    for fn in nc.m.functions:
        for blk in fn.blocks:
            il = blk.instructions
            if not any(
                i.sync_info is not None and len(i.sync_info.on_wait) > 1 for i in il
            ):
                continue
            new = []
            for ins in il:
                si = ins.sync_info
                waits = list(si.on_wait) if si is not None else []
                if len(waits) > 1:
                    si.on_wait = [waits[-1]]
                    for w in waits[:-1]:
                        nop = mybir.InstNoOp(
                            name=f"legalize-wait-{uid}",
                            engine=ins.engine,
                            sync_info=mybir.SyncInfo(on_wait=[w], on_update=[]),
                        )
                        uid += 1
                        new.append(nop)
                new.append(ins)
            blk.instructions = new


# ----------------------------------------------------------------------------
# device program builder
class _B:
    """Builder state."""

    def __init__(self, nc, tc, dram, sb, psum):
        self.nc, self.tc = nc, tc
        self.dram, self.sb, self.psum = dram, sb, psum
        self.uid = 0
        self.ident = None    # [P, P] identity f32
        self.epsrow = None   # [P, MB, D] eps*I row slab (per-core input)

    def u(self, s):
        self.uid += 1
        return f"{s}_{self.uid}"


def _stream_view(full_ap):
    """[D, D] dram AP -> [P, NCH, CH, D] chunked k-tile stream view."""
    return full_ap.rearrange("(ch kb p) n -> p ch kb n", p=P, kb=CH)


def _lhsT_view(a2a_ap):
    """[D, SH] dram AP (A2A out, flat) -> [P, KT, SH]."""
    return a2a_ap.rearrange("(k p) m -> p k m", p=P)


def _mm_shard(b: _B, lhsT_sb, rhs_chunks, scale, eps_coef, tag="ostag"):
    """out_stag[P, MB, D] = (lhsT^T @ rhs) * scale (+ eps_coef * epsrow).

    lhsT_sb: [P, KT, SH] f32 sbuf; rhs_chunks: [P, NCHUNK, CH, D] dram view.
    scale: float or AP. eps_coef: None or float g (adds g * epsrow).
    """
    nc = b.nc
    stag = b.sb.tile([P, MB, D], F32R, tag=tag, name=b.u(tag), bufs=_TAG_BUFS[tag])
    ps = [
        b.psum.tile([P, 512], F32, tag="mmps", name=b.u("ps"))
        for _ in range(MB * NB)
    ]
    for ch in range(KT // CH):
        rt = b.sb.tile([P, CH, D], F32R, tag="rstream", name=b.u("rt"), bufs=_TAG_BUFS["rstream"])
        nc.sync.dma_start(out=rt[:], in_=rhs_chunks[:, ch])
        for kk in range(CH):
            k = ch * CH + kk
            for m in range(MB):
                for n in range(NB):
                    nc.tensor.matmul(
                        ps[m * NB + n][:],
                        lhsT_sb[:, k, m * P:(m + 1) * P],
                        rt[:, kk, n * 512:(n + 1) * 512],
                        start=(k == 0),
                        stop=(k == KT - 1),
                    )
    for m in range(MB):
        for n in range(NB):
            if eps_coef is not None:
                # add (eps_coef/scale) * epsrow into psum pre-eviction so the
                # scaled eviction yields  scale*psum + eps_coef*epsrow
                nc.vector.scalar_tensor_tensor(
                    ps[m * NB + n][:],
                    b.epsrow[:, m, n * 512:(n + 1) * 512],
                    float(eps_coef) / _scale_const(scale),
                    ps[m * NB + n][:],
                    ALU.mult,
                    ALU.add,
                )
            nc.scalar.activation(
                stag[:, m, n * 512:(n + 1) * 512],
                ps[m * NB + n][:],
                AF.Copy,
                scale=scale,
            )
    return stag


def _scale_const(scale):
    assert isinstance(scale, (int, float)), "eps_coef requires constant scale"
    return float(scale)


def _transpose_shard(b: _B, stag):
    """[P, MB, D] staging (rows shard of X) -> [P, KT, SH] = X^T[:, shard cols]."""
    nc = b.nc
    tt = b.sb.tile([P, KT, SH], F32R, tag="lhsT", name=b.u("tt"), bufs=_TAG_BUFS["lhsT"])
    for k in range(KT):
        for m in range(MB):
            tp = b.psum.tile([P, 512], F32R, tag="mmps", name=b.u("tps"))
            nc.tensor.transpose(
                tp[:, 0:P], stag[:, m, k * P:(k + 1) * P], b.ident[:]
            )
            nc.scalar.copy(tt[:, k, m * P:(m + 1) * P], tp[:, 0:P])
    return tt


def _load_lhsT(b: _B, dram_flat_ap):
    """DMA [D, SH] dram -> [P, KT, SH] sbuf."""
    t = b.sb.tile([P, KT, SH], F32R, tag="lhsT", name=b.u("lh"), bufs=_TAG_BUFS["lhsT"])
    b.nc.sync.dma_start(out=t[:], in_=_lhsT_view(dram_flat_ap))
    return t


def _bounce_and_gather(b: _B, stag, want_a2a, name):
    """Write staging to DRAM, AllGather full (+ optionally AllToAll col-slice).

    Returns (full_dram_ap [D, D], a2a_out_ap [D, SH] or None).
    """
    nc = b.nc
    bounce = b.dram.tile([SH, D], F32R, name=b.u(f"bn_{name}"), tag="d_bn", bufs=4)
    nc.gpsimd.dma_start(
        out=bounce[:].rearrange("(m p) n -> p m n", p=P), in_=stag[:]
    )
    full = b.dram.tile([D, D], F32R, name=b.u(f"fl_{name}"), addr_space="Shared", tag="d_fl", bufs=4)
    nc.gpsimd.collective_compute(
        "AllGather",
        ALU.bypass,
        replica_groups=[list(range(NC))],
        ins=[bounce[:]],
        outs=[full[:]],
    )
    a2a_out = None
    if want_a2a:
        a2a_in = b.dram.tile([NC, SH, SH], F32R, name=b.u(f"ai_{name}"), tag="d_ai", bufs=4)
        for j in range(NC):
            nc.gpsimd.dma_start(
                out=a2a_in[j].rearrange("(m p) n -> p m n", p=P),
                in_=stag[:, :, j * SH:(j + 1) * SH],
            )
        a2a_out = b.dram.tile([NC * SH, SH], F32R, name=b.u(f"ao_{name}"), tag="d_ao", bufs=4)
        nc.gpsimd.collective_compute(
            "AllToAll",
            ALU.bypass,
            replica_groups=[list(range(NC))],
            ins=[a2a_in[:]],
            outs=[a2a_out[:]],
        )
    return full[:], (a2a_out[:] if a2a_out is not None else None)


def _ns_chain(b: _B, a_col_lhsT_sb, a_row_stag, sched, name):
    """Run a scaled NS chain. Inputs:
      a_col_lhsT_sb: [P, KT, SH] sbuf = A[:, shard cols]  (lhsT of A)
      a_row_stag:    [P, MB, D] sbuf = A[shard rows, :]   (row slab of A)
    Returns dict with Yfull, Zfull (dram APs), Y_a2a, Z_a2a, Y_stag (sbuf).
    """
    nc = b.nc
    al0, be0 = sched[0]
    # iter 1: T0 = al0*I + be0*A (sharded, elementwise); Z1 = T0; Y1 = A @ T0
    t0f = b.sb.tile([P, MB, D], F32, tag="f32tmp", name=b.u("t0f"), bufs=1)
    t0 = b.sb.tile([P, MB, D], F32R, tag="ostag", name=b.u("t0"), bufs=_TAG_BUFS["ostag"])
    for m in range(MB):
        nc.scalar.mul(t0f[:, m, :], a_row_stag[:, m, :].bitcast(F32), float(be0))
        nc.vector.scalar_tensor_tensor(
            t0f[:, m, :], b.epsrow[:, m, :], float(al0 / EPS),
            t0f[:, m, :], ALU.mult, ALU.add,
        )
        nc.scalar.copy(t0[:, m, :], t0f[:, m, :])
    t0_full, t0_a2a = _bounce_and_gather(b, t0, True, f"{name}t0")
    y_stag = _mm_shard(b, a_col_lhsT_sb, _stream_view(t0_full), 1.0, None)
    y_full, y_a2a = _bounce_and_gather(b, y_stag, True, f"{name}y1")
    st = dict(Yfull=y_full, Y_a2a=y_a2a, Zfull=t0_full, Z_a2a=t0_a2a, Y_stag=y_stag)

    for it in range(1, len(sched)):
        al, be = sched[it]
        lh_z = _get_lhsT(b, st, "Z")
        lh_y = _get_lhsT(b, st, "Y")
        # P = Z @ Y ; T = al*I + be*P  (keep T staging for local transpose)
        t_stag = _mm_shard(b, lh_z, _get_stream(b, st, "Y"), float(be), al / EPS,
                           tag="tstag")
        t_full, _ = _bounce_and_gather(b, t_stag, False, f"{name}t{it}")
        # Z' = T @ Z : lhsT = T^T[:, shard] = transpose of own T staging
        lh_tt = _transpose_shard(b, t_stag)
        z_stag = _mm_shard(b, lh_tt, _get_stream(b, st, "Z"), 1.0, None,
                           tag="zstag")
        # Y' = Y @ T
        y_stag = _mm_shard(b, lh_y, _stream_view(t_full), 1.0, None)
        # batched gather of (Y', Z')
        bounce = b.dram.tile([2 * SH, D], F32R, name=b.u("bnyz"), tag="d_bnyz", bufs=4)
        nc.gpsimd.dma_start(
            out=bounce[:].rearrange("(t m p) n -> t p m n", t=2, p=P)[0],
            in_=y_stag[:])
        nc.gpsimd.dma_start(
            out=bounce[:].rearrange("(t m p) n -> t p m n", t=2, p=P)[1],
            in_=z_stag[:])
        full = b.dram.tile([NC * 2 * SH, D], F32R, name=b.u("flyz"),
                           addr_space="Shared", tag="d_flyz", bufs=4)
        nc.gpsimd.collective_compute(
            "AllGather", ALU.bypass, replica_groups=[list(range(NC))],
            ins=[bounce[:]], outs=[full[:]],
        )
        a2a_in = b.dram.tile([NC, 2, SH, SH], F32R, name=b.u("aiyz"), tag="d_aiyz", bufs=4)
        for j in range(NC):
            nc.gpsimd.dma_start(
                out=a2a_in[j, 0].rearrange("(m p) n -> p m n", p=P),
                in_=y_stag[:, :, j * SH:(j + 1) * SH])
            nc.gpsimd.dma_start(
                out=a2a_in[j, 1].rearrange("(m p) n -> p m n", p=P),
                in_=z_stag[:, :, j * SH:(j + 1) * SH])
        a2a_out = b.dram.tile([NC, 2, SH, SH], F32R, name=b.u("aoyz"), tag="d_aoyz", bufs=4)
        nc.gpsimd.collective_compute(
            "AllToAll", ALU.bypass, replica_groups=[list(range(NC))],
            ins=[a2a_in[:]], outs=[a2a_out[:]],
        )
        # views: full rows = (c, t, m p); Y = t 0, Z = t 1
        fv = full[:].rearrange("(c t kb p) n -> t p c kb n", t=2, kb=CH, p=P)
        av = a2a_out[:].rearrange("s t (kb p) m -> t p s kb m", kb=CH, p=P)
        st = dict(
            Yfull=fv[0], Zfull=fv[1],           # [P, NC, CH, D] chunk views
            Y_a2a=av[0], Z_a2a=av[1],           # [P, s, kb, SH] 4d lhsT views
            Y_stag=y_stag, Z_stag=z_stag,
            chunked=True,
        )
    return st


def _load_lhsT4(b: _B, view4):
    """DMA [P, s, kb, SH] 4d view -> [P, KT, SH] sbuf (k = s*CH + kb)."""
    t = b.sb.tile([P, KT, SH], F32R, tag="lhsT", name=b.u("lh4"), bufs=_TAG_BUFS["lhsT"])
    for s in range(NC):
        b.nc.sync.dma_start(
            out=t[:, s * CH:(s + 1) * CH, :], in_=view4[:, s]
        )
    return t


def _get_lhsT(b, st, key):
    v = st[f"{key}_a2a"]
    if st.get("chunked"):
        return _load_lhsT4(b, v)
    return _load_lhsT(b, v)


def _get_stream(b, st, key):
    v = st[f"{key}full"]
    if st.get("chunked"):
        return v
    return _stream_view(v)


def build_device_program(k1, k2, repeat=1):
    sched1 = make_schedule(EPS, 1.0 + EPS, k1)
    sched2 = make_schedule(EPS, 1.0 + EPS, k2)

    nc = bass.Bass(num_devices=NC)
    with PatchedTileContext(nc) as tc:
        with tc.tile_pool(name="dram", bufs=1, space="DRAM") as dram, \
             tc.tile_pool(name="sb", bufs=1) as sb_const, \
             tc.tile_pool(name="sbw", bufs=3) as sbw, \
             tc.tile_pool(name="psum", bufs=8, space="PSUM") as psum:

            b = _B(nc, tc, dram, sbw, psum)

            # --- inputs
            a1col = dram.tile([D, SH], F32R, kind="ExternalInput", name="a1col", uniquify=False)
            a1row = dram.tile([SH, D], F32, kind="ExternalInput", name="a1row", uniquify=False)
            ctcol = dram.tile([D, SH], F32R, kind="ExternalInput", name="ctcol", uniquify=False)
            epsrow_d = dram.tile([SH, D], F32, kind="ExternalInput", name="epsrow", uniquify=False)
            invc2_d = dram.tile([P, 1], F32, kind="ExternalInput", name="invc2", uniquify=False)
            partials_d = dram.tile([P, 8], F32, kind="ExternalOutput", name="partials", uniquify=False)

            # --- constants resident in SBUF
            ident_f = sb_const.tile([P, P], F32, name="ident_f", uniquify=False)
            make_identity(nc, ident_f[:])
            ident = sb_const.tile([P, P], F32R, name="ident", uniquify=False)
            nc.scalar.copy(ident[:], ident_f[:])
            b.ident = ident
            epsrow = sb_const.tile([P, MB, D], F32, name="epsrow_sb", uniquify=False)
            nc.sync.dma_start(out=epsrow[:], in_=epsrow_d[:].rearrange("(m p) n -> p m n", p=P))
            b.epsrow = epsrow
            invc2 = sb_const.tile([P, 1], F32, name="invc2_sb", uniquify=False)
            nc.sync.dma_start(out=invc2[:], in_=invc2_d[:])
            part = sb_const.tile([P, 8], F32, name="part_sb", uniquify=False)
            b.part = part

            for _rep in range(repeat):
                _emit_pipeline(b, nc, sched1, sched2, a1col, a1row, ctcol,
                               epsrow, invc2, partials_d)

    legalize_single_wait(nc)
    return nc


def _emit_pipeline(b, nc, sched1, sched2, a1col, a1row, ctcol, epsrow, invc2,
                   partials_d):
    if True:
        if True:
            # --- NS1 on A1 (uploaded: Cp/c1 + eps I)
            a1c_sb = _load_lhsT(b, a1col[:])
            a1r_sb = b.sb.tile([P, MB, D], F32, tag="ostag", name="a1r_sb", bufs=_TAG_BUFS["ostag"])
            nc.sync.dma_start(out=a1r_sb[:], in_=a1row[:].rearrange("(m p) n -> p m n", p=P))
            st1 = _ns_chain(b, a1c_sb, a1r_sb, sched1, "n1")

            # --- NS1 half-step: S = Y*(1.5 I - 0.5 Z Y)
            lh_z = _get_lhsT(b, st1, "Z")
            lh_y = _get_lhsT(b, st1, "Y")
            tp_stag = _mm_shard(b, lh_z, _get_stream(b, st1, "Y"), -0.5, 1.5 / EPS,
                                tag="tstag")
            tp_full, _ = _bounce_and_gather(b, tp_stag, False, "half")
            s_stag = _mm_shard(b, lh_y, _stream_view(tp_full), 1.0, None)
            s_full, s_a2a = _bounce_and_gather(b, s_stag, True, "sfin")

            # --- middle: V = (Ct @ S)/c2 ; A2 = S @ V + eps I
            ct_sb = _load_lhsT(b, ctcol[:])
            v_stag = _mm_shard(b, ct_sb, _stream_view(s_full), invc2[:, 0:1],
                               None, tag="tstag")
            v_full, _ = _bounce_and_gather(b, v_stag, False, "vmid")
            lh_s = _load_lhsT(b, s_a2a)
            a2_stag = _mm_shard(b, lh_s, _stream_view(v_full), 1.0, 1.0)
            # A2: only A2A needed (lhsT for NS2 iter1); row slab is local staging
            a2a_in = b.dram.tile([NC, SH, SH], F32R, name=b.u("ai_a2"), tag="d_ai", bufs=4)
            for j in range(NC):
                nc.gpsimd.dma_start(
                    out=a2a_in[j].rearrange("(m p) n -> p m n", p=P),
                    in_=a2_stag[:, :, j * SH:(j + 1) * SH])
            a2_a2a = b.dram.tile([NC * SH, SH], F32R, name=b.u("ao_a2"), tag="d_ao", bufs=4)
            nc.gpsimd.collective_compute(
                "AllToAll", ALU.bypass, replica_groups=[list(range(NC))],
                ins=[a2a_in[:]], outs=[a2_a2a[:]],
            )
            a2c_sb = _load_lhsT(b, a2_a2a[:])

            # --- NS2
            st2 = _ns_chain(b, a2c_sb, a2_stag, sched2, "n2")

            # --- trace stage: U2 = Y2 @ Z2 (staging only)
            lh_y2 = _get_lhsT(b, st2, "Y")
            u2_stag = _mm_shard(b, lh_y2, _get_stream(b, st2, "Z"), 1.0, None,
                                tag="tstag")
            y2_stag = st2["Y_stag"]
            part = b.part
            nc.gpsimd.memset(part[:], 0.0)
            tmp = b.sb.tile([P, MB, D], F32, tag="f32tmp", name=b.u("tmp"), bufs=1)
            for m in range(MB):
                nc.vector.tensor_mul(
                    tmp[:, m, :], y2_stag[:, m, :].bitcast(F32),
                    u2_stag[:, m, :].bitcast(F32))
                nc.vector.tensor_reduce(
                    part[:, m:m + 1], tmp[:, m, :], mybir.AxisListType.X, ALU.add)
                nc.vector.tensor_mul(
                    tmp[:, m, :], y2_stag[:, m, :].bitcast(F32), epsrow[:, m, :])
                nc.vector.tensor_reduce(
                    part[:, 2 + m:3 + m], tmp[:, m, :], mybir.AxisListType.X, ALU.add)
            nc.sync.dma_start(out=partials_d[:], in_=part[:])


# ----------------------------------------------------------------------------
# host golden model (mirrors device pipeline exactly, fp32, no hw noise)
def golden_loss(predictions, targets, k1=K1, k2=K2):
    mu_p, Cp = _unpack_row(predictions[0])
    mu_t, Ct = _unpack_row(targets[0])
    c1 = _power_iter_sym(Cp) * 1.02
    c2 = _power_iter_prod(Cp, Ct) * 1.05 / c1
    I = np.eye(D, dtype=np.float32)
    A1 = (Cp / c1 + EPS * I).astype(np.float32)

    def chain(A, sched):
        al, be = sched[0]
        T0 = (al * I + be * A).astype(np.float32)
        Y, Z = A @ T0, T0
        for alk, bek in sched[1:]:
            Pm = Z @ Y
            T = alk * I + bek * Pm
            Y, Z = Y @ T, T @ Z
        return Y, Z

    Y1, Z1 = chain(A1, make_schedule(EPS, 1.0 + EPS, k1))
    S = Y1 @ (1.5 * I - 0.5 * (Z1 @ Y1))
    V = (Ct @ S) / c2
    A2 = (S @ V + EPS * I).astype(np.float32)
    Y2, Z2 = chain(A2, make_schedule(EPS, 1.0 + EPS, k2))
    U2 = Y2 @ Z2
    tr_corr = 1.5 * np.trace(Y2.astype(np.float64)) - 0.5 * float(
        np.sum(Y2.astype(np.float64) * U2.astype(np.float64)))
    tr_sqrtM = np.sqrt(c1 * c2) * tr_corr
    mu_term = float(np.mean((mu_p - mu_t) ** 2))
    return np.float32(mu_term + np.trace(Cp.astype(np.float64))
                      + np.trace(Ct.astype(np.float64)) + 2.0 * tr_sqrtM)


# ----------------------------------------------------------------------------
# entry point
def _get_program():
    key = (K1, K2)
    if key not in _BUILD_CACHE:
        _BUILD_CACHE[key] = build_device_program(K1, K2)
    return _BUILD_CACHE[key]


_RUNNER = None
_DIAG_PACKED = None


def _get_runner():
    """Build the device program and a HELD jit callable once per process.

    run_bass_kernel_spmd constructs a fresh closure + jax.jit per call, which
    forces a full retrace/re-lower of the (large) Bass module every time
    (3-14s/call measured).  This mirrors its axon path (bass2jax) but keeps
    one jitted executable and pre-uploads the call-invariant epsrow input.
    """
    global _RUNNER
    if _RUNNER is not None:
        return _RUNNER

    import jax
    from jax.sharding import Mesh, PartitionSpec, NamedSharding
    from jax.experimental.shard_map import shard_map
    from concourse import bass2jax

    nc = _get_program()
    bass2jax.install_neuronx_cc_hook()
    partition_name = (nc.partition_id_tensor.name
                      if nc.partition_id_tensor is not None else None)
    in_names, out_names, out_avals, zero_shapes = [], [], [], []
    for alloc in nc.m.functions[0].allocations:
        if not isinstance(alloc, mybir.MemoryLocationSet):
            continue
        name = alloc.memorylocations[0].name
        if alloc.kind == "ExternalInput":
            if name != partition_name:
                in_names.append(name)
        elif alloc.kind == "ExternalOutput":
            shape = tuple(alloc.tensor_shape)
            dtype = mybir.dt.np(alloc.dtype)
            out_avals.append(jax.core.ShapedArray(shape, dtype))
            out_names.append(name)
            zero_shapes.append((shape, dtype))
    n_params = len(in_names)
    n_outs = len(out_avals)
    in_names_all = list(in_names) + out_names
    if partition_name is not None:
        in_names_all.append(partition_name)
    donate = tuple(range(n_params, n_params + n_outs))

    def _body(*args):
        operands = list(args)
        if partition_name is not None:
            operands.append(bass2jax.partition_id_tensor())
        outs = bass2jax._bass_exec_p.bind(
            *operands,
            out_avals=tuple(out_avals),
            in_names=tuple(in_names_all),
            out_names=tuple(out_names),
            lowering_input_output_aliases=(),
            sim_require_finite=True,
            sim_require_nnan=True,
            nc=nc,
        )
        return tuple(outs)

    devices = jax.devices()[:NC]
    mesh = Mesh(np.asarray(devices), ("core",))
    sharding = NamedSharding(mesh, PartitionSpec("core"))
    in_specs = (PartitionSpec("core"),) * (n_params + n_outs)
    out_specs = (PartitionSpec("core"),) * len(out_names)
    sharded = jax.jit(
        shard_map(_body, mesh=mesh, in_specs=in_specs, out_specs=out_specs,
                  check_rep=False),
        donate_argnums=donate,
        keep_unused=True,
    )

    # epsrow is call-invariant: upload it once and reuse the device array
    # (inputs are not donated, only the zero output buffers are).
    eps_np = np.zeros((NC * SH, D), np.float32)
    eps_np[np.arange(NC * SH), np.arange(NC * SH)] = EPS
    eps_dev = jax.device_put(eps_np, sharding)

    _RUNNER = dict(sharded=sharded, in_names=in_names,
                   zero_shapes=zero_shapes, sharding=sharding,
                   eps_dev=eps_dev)
    return _RUNNER


def kernel(predictions, targets):
    import jax
    global _DIAG_PACKED
    predictions = np.asarray(predictions)
    targets = np.asarray(targets)

    runner = _get_runner()
    put = lambda x: jax.device_put(x, runner["sharding"])

    # -- Cp first so its uploads overlap the rest of host prep
    mu_p, Cp = _unpack_row(predictions[0])
    c1 = _power_iter_sym(Cp) * 1.05
    A1 = (Cp / c1).astype(np.float32)
    A1[np.arange(D), np.arange(D)] += EPS
    a1row_dev = put(A1)                                   # row slabs = A1 itself
    a1col_dev = put(np.concatenate(
        [A1[:, c * SH:(c + 1) * SH] for c in range(NC)], axis=0))

    mu_t, Ct = _unpack_row(targets[0])
    ctcol_dev = put(np.concatenate(
        [Ct[:, c * SH:(c + 1) * SH] for c in range(NC)], axis=0))

    c2 = _power_iter_prod(Cp, Ct) * 1.10 / c1
    invc2_dev = put(np.full((NC * P, 1), 1.0 / c2, np.float32))

    # traces straight from the packed rows (diag of C sits at D + offsets)
    if _DIAG_PACKED is None:
        off = np.concatenate([[0], np.cumsum(np.arange(D, 1, -1))]).astype(np.int64)
        _DIAG_PACKED = D + off
    trCp = float(predictions[0][_DIAG_PACKED].sum(dtype=np.float64))
    trCt = float(targets[0][_DIAG_PACKED].sum(dtype=np.float64))
    mu_term = float(np.mean((mu_p - mu_t) ** 2))

    args = {"a1col": a1col_dev, "a1row": a1row_dev, "ctcol": ctcol_dev,
            "epsrow": runner["eps_dev"], "invc2": invc2_dev}
    zeros = [np.zeros((NC * s[0], *s[1:]), dt)
             for (s, dt) in runner["zero_shapes"]]
    out_arrs = runner["sharded"](
        *[args[nm] for nm in runner["in_names"]], *zeros)
    parts = np.asarray(out_arrs[0]).reshape(NC, P, 8)

    syu = float(parts[:, :, 0:2].sum(dtype=np.float64))
    trY2 = float(parts[:, :, 2:4].sum(dtype=np.float64)) / EPS
    tr_corr = 1.5 * trY2 - 0.5 * syu
    tr_sqrtM = np.sqrt(c1 * c2) * tr_corr

    loss = mu_term + trCp + trCt + 2.0 * tr_sqrtM
    return np.float32(loss)

